# revision 34
# baseline (speedup 1.0000x reference)
"""Trainium2 Bass kernel for windowless 3D relative-position attention.

Full-input contract: kernel(**inputs) takes the unsharded numpy inputs and
returns the full [4, 2048, 256] output. Internally shards across 8 NeuronCores
as (batch b = core//2) x (head-group g = core%2, 4 heads each).

v2: ACT-paced softmax pipeline. Per q-chunk (512 queries), the m-loop runs in
(m-block x head-pair) units: 2 row-packed K=32 score matmuls -> one N=1024
ACT exp (PSUM->SBUF fp16) -> one per-m N=2048 DVE multiply by the
host-precomputed exp(bias) -> 2 col-tiled M=33 AV matmuls accumulating
[v|1].T @ aw into per-pair PSUM banks (row 32/96 = softmax denominator).
PSUM budget: 4 banks score double-buffer + 2 AV accumulators + 2 for the
overlapped projection of the previous q-chunk.

Normalization avoids the per-head [1,512] DVE reciprocal (3.3us each): the AV
accumulators are copied to SBUF fp16, the 4 denominator rows are DMA-gathered
onto 4 partitions, and a single [4,512] reciprocal + broadcast-DMA + [32,512]
fp16 multiplies produce the normalized heads, which the output projection
consumes during the next q-chunk.

The bias add is factored through the exponential: exp(s+bias) =
exp(s)*exp(bias), with exp(bias.T - C_SHIFT) precomputed on host in fp16
(C_SHIFT keeps products in fp16 range; it cancels in the softmax ratio).
"""

import os
import sys
from contextlib import ExitStack

import numpy as np

sys.path.insert(0, "/opt/trn_rl_repo")

import concourse.bass as bass
import concourse.bacc as bacc
import concourse.tile as tile
from concourse import mybir
from concourse.bass_utils import run_bass_kernel_spmd

# Problem constants (hardcoded per contract)
B = 4
N = 2048
INP = 256
OUP = 256
HEADS = 8
DIM_HEAD = 32
SCALE = DIM_HEAD ** -0.5
HL = 4            # heads per core
MT = N // 128     # 16 m-tiles (keys)
NQ = 4            # 512-wide n (query) chunks
NQW = 512
C_SHIFT = 4.0

f32 = mybir.dt.float32
f16 = mybir.dt.float16

_LAST = {"exec_time_ns": None}


def _build_nc():
    nc = bacc.Bacc("TRN2", target_bir_lowering=False, debug=False)
    xT_d = nc.dram_tensor("xT", [2, 128, N], f16, kind="ExternalInput")
    wqk_d = nc.dram_tensor("w_qk", [2, 128, 256], f16, kind="ExternalInput")
    wv_d = nc.dram_tensor("w_v", [2, 128, 128], f16, kind="ExternalInput")
    # w_out rows per head, staged host-side as [4, 32, 256]
    wout_d = nc.dram_tensor("w_out4", [4, 32, 256], f16, kind="ExternalInput")
    ebt_d = nc.dram_tensor("expbt", [N, N], f16, kind="ExternalInput")
    out_d = nc.dram_tensor("partial", [N, OUP], f32, kind="ExternalOutput")
    # scratch: denominator gather + reciprocal broadcast staging
    den_d = nc.dram_tensor("den_scratch", [NQ, HL, NQW], f16)
    rec_d = nc.dram_tensor("rec_scratch", [NQ, HL, NQW], f16)

    with ExitStack() as ctx:
        tc = ctx.enter_context(tile.TileContext(nc))
        consts = ctx.enter_context(tc.tile_pool(name="consts", bufs=1))

        xT = consts.tile([128, 2, N], f16)
        wqk = consts.tile([128, 2, 256], f16)
        wv = consts.tile([128, 2, 128], f16)
        wout4 = consts.tile([128, HL, 256], f16)  # per-head rows at 0:32
        qkT = consts.tile([128, 2, N], f16)       # [:,0,:]=qT  [:,1,:]=kT
        vsb = consts.tile([128, MT, HL, 33], f16)  # [m%128, mtile, head, d|ones]
        aoutu = consts.tile([33, NQ, HL, NQW], f16)  # unnormalized AV + den row
        aoutT = consts.tile([32, HL, N], f16)     # normalized heads
        rden = consts.tile([4, NQ, NQW], f16)     # gathered denominators
        rrec = consts.tile([4, NQ, NQW], f16)     # reciprocals
        rb = consts.tile([32, NQ, HL, NQW], f16)  # broadcast reciprocals
        ostg = consts.tile([128, MT, OUP], f32)

        for kk in range(2):
            nc.sync.dma_start(out=xT[:, kk, :], in_=xT_d[kk])
            nc.sync.dma_start(out=wqk[:, kk, :], in_=wqk_d[kk])
            nc.sync.dma_start(out=wv[:, kk, :], in_=wv_d[kk])
        for hl in range(HL):
            nc.sync.dma_start(out=wout4[0:32, hl, :], in_=wout_d[hl])
        nc.vector.memset(vsb[:], 1.0)

        # --- q/k projection (transposed orientation) and v (natural) ---
        with tc.tile_pool(name="pps", bufs=4, space="PSUM") as pps:
            for mb in range(2):           # 0 -> q block, 1 -> k block
                for ch in range(NQ):
                    ps = pps.tile([128, 512], f32, tag="qkps")
                    for kk in range(2):
                        nc.tensor.matmul(
                            ps[:],
                            lhsT=wqk[:, kk, mb * 128:(mb + 1) * 128],
                            rhs=xT[:, kk, ch * 512:(ch + 1) * 512],
                            start=(kk == 0), stop=(kk == 1),
                        )
                    nc.vector.tensor_copy(
                        out=qkT[:, mb, ch * 512:(ch + 1) * 512], in_=ps[:]
                    )
            for nt in range(MT):
                ps = pps.tile([128, 128], f32, tag="vps")
                for kk in range(2):
                    nc.tensor.matmul(
                        ps[:],
                        lhsT=xT[:, kk, nt * 128:(nt + 1) * 128],
                        rhs=wv[:, kk, :],
                        start=(kk == 0), stop=(kk == 1),
                    )
                # scatter the 4 heads' 32 columns into the 33-wide vsb slots;
                # column 32 keeps the memset 1.0 (softmax denominator row)
                nc.vector.tensor_copy(out=vsb[:, nt, :, 0:32], in_=ps[:])

        # wout4c: DVE-owned copy so the projection matmul's weight dep is on
        # DVE (not a DMA sem) — matmuls may carry at most 2 sync waits
        wout4c = consts.tile([128, HL, 256], f16)
        nc.vector.tensor_copy(out=wout4c[0:32, :, :], in_=wout4[0:32, :, :])

        # --- attention: ACT-paced pipeline over (q, m, head-pair) units ---
        def normalize(q):
            """Batched-reciprocal softmax normalization for chunk q.
            Issued right after q's m-loop; overlaps the next q's m-loop."""
            # gather the 4 denominator rows (partition 32 of each head's
            # accumulator) onto 4 partitions via a DRAM bounce
            nc.sync.dma_start(out=den_d[q].unsqueeze(0),
                              in_=aoutu[32:33, q, :, :])
            nc.sync.dma_start(out=rden[:, q, :], in_=den_d[q])
            with nc.allow_low_precision(reason="fp16 1/den; 5e-4 rel ok"):
                nc.vector.reciprocal(out=rrec[:, q, :], in_=rden[:, q, :])
            nc.sync.dma_start(out=rec_d[q], in_=rrec[:, q, :])
            # broadcast each head's reciprocal to 32 partitions and multiply
            for hl in range(HL):
                dst = rec_d[q, hl]
                src_b = bass.AP(
                    tensor=dst.tensor, offset=dst.offset,
                    ap=[[0, 32], dst.ap[-1]],
                )
                nc.sync.dma_start(out=rb[:, q, hl, :], in_=src_b)
            for hl in range(HL):
                nc.vector.tensor_mul(
                    aoutT[:, hl, q * NQW:(q + 1) * NQW],
                    aoutu[0:32, q, hl, :],
                    rb[:, q, hl, :],
                )

        with tc.tile_pool(name="sps", bufs=2, space="PSUM") as sps, \
             tc.tile_pool(name="awp", bufs=3) as awp, \
             tc.tile_pool(name="aw2p", bufs=3) as aw2p, \
             tc.tile_pool(name="ebtp", bufs=3) as ebtp:
          with tc.tile_pool(name="oap", bufs=1, space="PSUM") as oap:
            for q in range(NQ):
                ncol0 = q * NQW
                oa = [oap.tile([33, NQW], f32, tag=f"oa{i}",
                               name=f"oa{i}_{q}") for i in range(HL)]
                for m in range(MT):
                    ebt = ebtp.tile([128, NQW], f16)
                    nc.sync.dma_start(
                        out=ebt[:],
                        in_=ebt_d[m * 128:(m + 1) * 128, ncol0:ncol0 + NQW],
                    )
                    aw = awp.tile([128, HL, NQW], f16)
                    for hp in range(2):
                        # 2 heads row-packed; separate PSUM banks per head
                        sc = sps.tile([128, 2, NQW], f32)
                        for hi in range(2):
                            hl = hp * 2 + hi
                            nc.tensor.matmul(
                                sc[:, hi, :],
                                lhsT=qkT[32 * hl:32 * (hl + 1), 1,
                                         m * 128:(m + 1) * 128],
                                rhs=qkT[32 * hl:32 * (hl + 1), 0,
                                        ncol0:ncol0 + NQW],
                                start=True, stop=True,
                                tile_position=(32 * hl, 0),
                            )
                        nc.scalar.activation(
                            out=aw[:, 2 * hp:2 * hp + 2, :], in_=sc[:],
                            func=mybir.ActivationFunctionType.Exp,
                            scale=SCALE,
                        )
                    # one wide fp16 multiply for all 4 heads of this m-block
                    eb_b = bass.AP(
                        tensor=ebt.tensor, offset=ebt.offset,
                        ap=[ebt.ap[0], [0, HL], ebt.ap[1]],
                    )
                    aw2 = aw2p.tile([128, HL, NQW], f16)
                    nc.vector.tensor_mul(aw2[:], aw[:], eb_b)
                    # AV: M=33 matmul per head into its own PSUM bank
                    # (row 32 = softmax denominator via the vsb ones column)
                    for hl in range(HL):
                        nc.tensor.matmul(
                            oa[hl][:, :],
                            lhsT=vsb[:, m, hl, :],
                            rhs=aw2[:, hl, :],
                            start=(m == 0), stop=(m == MT - 1),
                        )
                # stash unnormalized AV output (frees the oa banks)
                for hl in range(HL):
                    nc.vector.tensor_copy(
                        out=aoutu[:, q, hl, :], in_=oa[hl][:]
                    )
                normalize(q)

          # output projection (inside sps's scope, on the freed oap banks)
          with tc.tile_pool(name="prj", bufs=4, space="PSUM") as prj:
            for nb in range(MT):
                pp = prj.tile([128, OUP], f32)
                for hl in range(HL):
                    nc.tensor.matmul(
                        pp[:],
                        lhsT=aoutT[:, hl, nb * 128:(nb + 1) * 128],
                        rhs=wout4c[0:32, hl, :],
                        start=(hl == 0), stop=(hl == HL - 1),
                    )
                nc.vector.tensor_copy(out=ostg[:, nb, :], in_=pp[:])
                nc.sync.dma_start(
                    out=out_d[nb * 128:(nb + 1) * 128, :],
                    in_=ostg[:, nb, :],
                )
    nc.compile()
    return nc


_NC_CACHE = {}


def kernel(x, w_qkv, bias_table, w_out, b_out, relative_pos):
    x = np.asarray(x, np.float32)
    w_qkv = np.asarray(w_qkv, np.float32)
    bias_table = np.asarray(bias_table, np.float32)
    w_out = np.asarray(w_out, np.float32)
    b_out = np.asarray(b_out, np.float32)
    relative_pos = np.asarray(relative_pos, np.int32)

    bias = bias_table[relative_pos, 0]                       # [n, m]
    expBT = np.exp(bias.T - C_SHIFT).astype(np.float16)      # [m, n]
    expBT = np.ascontiguousarray(expBT)

    if "nc" not in _NC_CACHE:
        _NC_CACHE["nc"] = _build_nc()
    nc = _NC_CACHE["nc"]

    in_maps = []
    for c in range(8):
        b, g = c // 2, c % 2
        w_qk = np.concatenate(
            [w_qkv[:, g * 128:(g + 1) * 128],
             w_qkv[:, 256 + g * 128:256 + (g + 1) * 128]], axis=1)
        in_maps.append({
            "xT": np.ascontiguousarray(x[b].T).reshape(2, 128, N).astype(np.float16),
            "w_qk": np.ascontiguousarray(w_qk).reshape(2, 128, 256).astype(np.float16),
            "w_v": np.ascontiguousarray(
                w_qkv[:, 512 + g * 128:512 + (g + 1) * 128]
            ).reshape(2, 128, 128).astype(np.float16),
            "w_out4": np.ascontiguousarray(
                w_out[g * 128:(g + 1) * 128, :]
            ).reshape(4, 32, 256).astype(np.float16),
            "expbt": expBT,
        })

    trace = bool(os.environ.get("KERNEL_TRACE"))
    res = run_bass_kernel_spmd(nc, in_maps, list(range(8)), trace=trace)
    _LAST["exec_time_ns"] = res.exec_time_ns
    _LAST["results"] = res

    parts = [np.asarray(res.results[c]["partial"], np.float32) for c in range(8)]
    out = np.stack([parts[2 * b] + parts[2 * b + 1] + b_out for b in range(B)])
    return out.astype(np.float32)


# revision 42
# speedup vs baseline: 1.0077x; 1.0077x over previous
"""Trainium2 Bass kernel for windowless 3D relative-position attention.

Full-input contract: kernel(**inputs) takes the unsharded numpy inputs and
returns the full [4, 2048, 256] output. Internally shards across 8 NeuronCores
as (batch b = core//2) x (head-group g = core%2, 4 heads each).

v2: ACT-paced softmax pipeline. Per q-chunk (512 queries), the m-loop runs in
(m-block x head-pair) units: 2 row-packed K=32 score matmuls -> one N=1024
ACT exp (PSUM->SBUF fp16) -> one per-m N=2048 DVE multiply by the
host-precomputed exp(bias) -> 2 col-tiled M=33 AV matmuls accumulating
[v|1].T @ aw into per-pair PSUM banks (row 32/96 = softmax denominator).
PSUM budget: 4 banks score double-buffer + 2 AV accumulators + 2 for the
overlapped projection of the previous q-chunk.

Normalization avoids the per-head [1,512] DVE reciprocal (3.3us each): the AV
accumulators are copied to SBUF fp16, the 4 denominator rows are DMA-gathered
onto 4 partitions, and a single [4,512] reciprocal + broadcast-DMA + [32,512]
fp16 multiplies produce the normalized heads, which the output projection
consumes during the next q-chunk.

The bias add is factored through the exponential: exp(s+bias) =
exp(s)*exp(bias), with exp(bias.T - C_SHIFT) precomputed on host in fp16
(C_SHIFT keeps products in fp16 range; it cancels in the softmax ratio).
"""

import os
import sys
from contextlib import ExitStack

import numpy as np

sys.path.insert(0, "/opt/trn_rl_repo")

import concourse.bass as bass
import concourse.bacc as bacc
import concourse.tile as tile
from concourse import mybir
from concourse.bass_utils import run_bass_kernel_spmd

# Problem constants (hardcoded per contract)
B = 4
N = 2048
INP = 256
OUP = 256
HEADS = 8
DIM_HEAD = 32
SCALE = DIM_HEAD ** -0.5
HL = 4            # heads per core
MT = N // 128     # 16 m-tiles (keys)
NQ = 4            # 512-wide n (query) chunks
NQW = 512
C_SHIFT = 4.0

f32 = mybir.dt.float32
f16 = mybir.dt.float16

_LAST = {"exec_time_ns": None}


def _build_nc():
    nc = bacc.Bacc("TRN2", target_bir_lowering=False, debug=False)
    xT_d = nc.dram_tensor("xT", [2, 128, N], f16, kind="ExternalInput")
    wqk_d = nc.dram_tensor("w_qk", [2, 128, 256], f16, kind="ExternalInput")
    wv_d = nc.dram_tensor("w_v", [2, 128, 128], f16, kind="ExternalInput")
    # w_out rows per head, staged host-side as [4, 32, 256]
    wout_d = nc.dram_tensor("w_out4", [4, 32, 256], f16, kind="ExternalInput")
    ebt_d = nc.dram_tensor("expbt", [N, N], f16, kind="ExternalInput")
    out_d = nc.dram_tensor("partial", [N, OUP], f32, kind="ExternalOutput")
    # scratch: denominator gather + reciprocal broadcast staging
    den_d = nc.dram_tensor("den_scratch", [NQ, HL, NQW], f16)
    rec_d = nc.dram_tensor("rec_scratch", [NQ, HL, NQW], f16)

    with ExitStack() as ctx:
        tc = ctx.enter_context(tile.TileContext(nc))
        consts = ctx.enter_context(tc.tile_pool(name="consts", bufs=1))

        xT = consts.tile([128, 2, N], f16)
        wqk = consts.tile([128, 2, 256], f16)
        wv = consts.tile([128, 2, 128], f16)
        # head-pair "slot" layout: slot s holds heads 2s (rows 0:32) and
        # 2s+1 (rows 64:96); softmax denominator rows land on 32 / 96
        wout4 = consts.tile([128, 2, 256], f16)
        qkT = consts.tile([128, 2, N], f16)       # [:,0,:]=qT  [:,1,:]=kT
        vsb = consts.tile([128, MT, HL, 33], f16)  # [m%128, mtile, head, d|ones]
        aoutu = consts.tile([128, NQ, 2, NQW], f16)  # unnormalized AV + dens
        aoutT = consts.tile([128, 2, N], f16)     # normalized heads
        rden = consts.tile([4, NQ, NQW], f16)     # gathered denominators
        rrec = consts.tile([4, NQ, NQW], f16)     # reciprocals
        rb = consts.tile([128, NQ, 2, NQW], f16)  # broadcast reciprocals
        ostg = consts.tile([128, MT, OUP], f32)

        for kk in range(2):
            nc.sync.dma_start(out=xT[:, kk, :], in_=xT_d[kk])
            nc.sync.dma_start(out=wqk[:, kk, :], in_=wqk_d[kk])
            nc.sync.dma_start(out=wv[:, kk, :], in_=wv_d[kk])
        # zero-fill BEFORE the DMAs so the whole-tile wout4c copy below reads
        # fully-initialized memory without clobbering the weights
        nc.vector.memset(wout4[:], 0.0)
        for hl in range(HL):
            nc.sync.dma_start(
                out=wout4[64 * (hl % 2):64 * (hl % 2) + 32, hl // 2, :],
                in_=wout_d[hl],
            )
        nc.vector.memset(vsb[:], 1.0)

        # --- q/k projection (transposed orientation) and v (natural) ---
        with tc.tile_pool(name="pps", bufs=4, space="PSUM") as pps:
            for mb in range(2):           # 0 -> q block, 1 -> k block
                for ch in range(NQ):
                    ps = pps.tile([128, 512], f32, tag="qkps")
                    for kk in range(2):
                        nc.tensor.matmul(
                            ps[:],
                            lhsT=wqk[:, kk, mb * 128:(mb + 1) * 128],
                            rhs=xT[:, kk, ch * 512:(ch + 1) * 512],
                            start=(kk == 0), stop=(kk == 1),
                        )
                    nc.vector.tensor_copy(
                        out=qkT[:, mb, ch * 512:(ch + 1) * 512], in_=ps[:]
                    )
            for nt in range(MT):
                ps = pps.tile([128, 128], f32, tag="vps")
                for kk in range(2):
                    nc.tensor.matmul(
                        ps[:],
                        lhsT=xT[:, kk, nt * 128:(nt + 1) * 128],
                        rhs=wv[:, kk, :],
                        start=(kk == 0), stop=(kk == 1),
                    )
                # scatter the 4 heads' 32 columns into the 33-wide vsb slots;
                # column 32 keeps the memset 1.0 (softmax denominator row)
                nc.vector.tensor_copy(out=vsb[:, nt, :, 0:32], in_=ps[:])

        # wout4c: DVE-owned copy so the projection matmul's weight dep is on
        # DVE (not a DMA sem) — matmuls may carry at most 2 sync waits
        wout4c = consts.tile([128, 2, 256], f16)
        nc.vector.tensor_copy(out=wout4c[:], in_=wout4[:])

        # --- attention: ACT-paced pipeline over (q, m, head-pair) units ---
        def normalize_and_project(prj, q):
            """Batched-reciprocal normalization + output projection for
            chunk q. Issued right after q's m-loop; overlaps chunk q+1."""
            # gather the 4 denominator rows (partitions 32/96 of each pair
            # bank) onto 4 partitions via a DRAM bounce — one plain
            # single-partition DMA per head
            for hl in range(HL):
                base, slot = 64 * (hl % 2), hl // 2
                nc.sync.dma_start(
                    out=den_d[q, hl],
                    in_=aoutu[base + 32:base + 33, q, slot, :],
                )
            nc.sync.dma_start(out=rden[:, q, :], in_=den_d[q])
            with nc.allow_low_precision(reason="fp16 1/den; 5e-4 rel ok"):
                nc.vector.reciprocal(out=rrec[:, q, :], in_=rden[:, q, :])
            nc.sync.dma_start(out=rec_d[q], in_=rrec[:, q, :])
            # broadcast each head's reciprocal to 32 partitions and multiply
            for hl in range(HL):
                base, slot = 64 * (hl % 2), hl // 2
                dst = rec_d[q, hl]
                src_b = bass.AP(
                    tensor=dst.tensor, offset=dst.offset,
                    ap=[[0, 32], dst.ap[-1]],
                )
                nc.sync.dma_start(out=rb[base:base + 32, q, slot, :],
                                  in_=src_b)
            for hl in range(HL):
                base, slot = 64 * (hl % 2), hl // 2
                nc.vector.tensor_mul(
                    aoutT[base:base + 32, slot, q * NQW:(q + 1) * NQW],
                    aoutu[base:base + 32, q, slot, :],
                    rb[base:base + 32, q, slot, :],
                )
            # projection: heads at partition bases 0 vs 64 run in different
            # PE row groups (concurrent!) so they accumulate into separate
            # banks, summed on DVE into the staging tile
            for nb in range(4 * q, 4 * (q + 1)):
                pp = [prj.tile([128, OUP], f32, tag=f"pp{j}",
                               name=f"pp{j}_{nb}") for j in range(2)]
                for hl in range(HL):
                    base, slot = 64 * (hl % 2), hl // 2
                    nc.tensor.matmul(
                        pp[hl % 2][:],
                        lhsT=aoutT[base:base + 32, slot,
                                   nb * 128:(nb + 1) * 128],
                        rhs=wout4c[base:base + 32, slot, :],
                        start=(hl < 2), stop=(hl >= 2),
                        tile_position=(base, 0),
                    )
                # two steps: a TensorTensor may read only ONE input from PSUM
                nc.vector.tensor_copy(out=ostg[:, nb, :], in_=pp[0][:])
                nc.vector.tensor_add(ostg[:, nb, :], ostg[:, nb, :], pp[1][:])

        with tc.tile_pool(name="sps", bufs=2, space="PSUM") as sps, \
             tc.tile_pool(name="oap", bufs=1, space="PSUM") as oap, \
             tc.tile_pool(name="prj", bufs=1, space="PSUM") as prj, \
             tc.tile_pool(name="awp", bufs=3) as awp, \
             tc.tile_pool(name="aw2p", bufs=3) as aw2p, \
             tc.tile_pool(name="ebtp", bufs=3) as ebtp:
            for q in range(NQ):
                ncol0 = q * NQW
                oa = [oap.tile([128, NQW], f32, tag=f"oa{i}",
                               name=f"oa{i}_{q}") for i in range(2)]
                # zero the pair banks; AV matmuls accumulate with
                # start=False, which is order-free regardless of stale
                # per-element has_written state (add-onto-0 == overwrite)
                for slot in range(2):
                    nc.vector.memset(oa[slot][:], 0.0)
                for m in range(MT):
                    ebt = ebtp.tile([128, NQW], f16)
                    nc.sync.dma_start(
                        out=ebt[:],
                        in_=ebt_d[m * 128:(m + 1) * 128, ncol0:ncol0 + NQW],
                    )
                    aw = awp.tile([128, HL, NQW], f16)
                    for hp in range(2):
                        # 2 heads row-packed; separate PSUM banks per head
                        sc = sps.tile([128, 2, NQW], f32)
                        for hi in range(2):
                            hl = hp * 2 + hi
                            nc.tensor.matmul(
                                sc[:, hi, :],
                                lhsT=qkT[32 * hl:32 * (hl + 1), 1,
                                         m * 128:(m + 1) * 128],
                                rhs=qkT[32 * hl:32 * (hl + 1), 0,
                                        ncol0:ncol0 + NQW],
                                start=True, stop=True,
                                tile_position=(32 * hl, 0),
                            )
                        nc.scalar.activation(
                            out=aw[:, 2 * hp:2 * hp + 2, :], in_=sc[:],
                            func=mybir.ActivationFunctionType.Exp,
                            scale=SCALE,
                        )
                    # one wide fp16 multiply for all 4 heads of this m-block
                    eb_b = bass.AP(
                        tensor=ebt.tensor, offset=ebt.offset,
                        ap=[ebt.ap[0], [0, HL], ebt.ap[1]],
                    )
                    aw2 = aw2p.tile([128, HL, NQW], f16)
                    nc.vector.tensor_mul(aw2[:], aw[:], eb_b)
                    # AV: col-tiled M=33 per head; pair (2s, 2s+1) lands on
                    # partitions 0:33 / 64:97 of slot s's bank (row 32/96 =
                    # softmax denominator via the vsb ones column)
                    for hl in range(HL):
                        base, slot = 64 * (hl % 2), hl // 2
                        nc.tensor.matmul(
                            oa[slot][base:base + 33, :],
                            lhsT=vsb[:, m, hl, :],
                            rhs=aw2[:, hl, :],
                            start=False, stop=(m == MT - 1),
                            skip_group_check=True,
                            tile_position=(0, base),
                        )
                # stash unnormalized AV output (frees the oa banks)
                for slot in range(2):
                    for base in (0, 64):
                        nc.vector.tensor_copy(
                            out=aoutu[base:base + 33, q, slot, :],
                            in_=oa[slot][base:base + 33, :],
                        )
                normalize_and_project(prj, q)

        nc.sync.dma_start(
            out=out_d[:].rearrange("(t p) o -> p t o", p=128), in_=ostg[:]
        )
    nc.compile()
    return nc


_NC_CACHE = {}


def kernel(x, w_qkv, bias_table, w_out, b_out, relative_pos):
    x = np.asarray(x, np.float32)
    w_qkv = np.asarray(w_qkv, np.float32)
    bias_table = np.asarray(bias_table, np.float32)
    w_out = np.asarray(w_out, np.float32)
    b_out = np.asarray(b_out, np.float32)
    relative_pos = np.asarray(relative_pos, np.int32)

    bias = bias_table[relative_pos, 0]                       # [n, m]
    expBT = np.exp(bias.T - C_SHIFT).astype(np.float16)      # [m, n]
    expBT = np.ascontiguousarray(expBT)

    if "nc" not in _NC_CACHE:
        _NC_CACHE["nc"] = _build_nc()
    nc = _NC_CACHE["nc"]

    in_maps = []
    for c in range(8):
        b, g = c // 2, c % 2
        w_qk = np.concatenate(
            [w_qkv[:, g * 128:(g + 1) * 128],
             w_qkv[:, 256 + g * 128:256 + (g + 1) * 128]], axis=1)
        in_maps.append({
            "xT": np.ascontiguousarray(x[b].T).reshape(2, 128, N).astype(np.float16),
            "w_qk": np.ascontiguousarray(w_qk).reshape(2, 128, 256).astype(np.float16),
            "w_v": np.ascontiguousarray(
                w_qkv[:, 512 + g * 128:512 + (g + 1) * 128]
            ).reshape(2, 128, 128).astype(np.float16),
            "w_out4": np.ascontiguousarray(
                w_out[g * 128:(g + 1) * 128, :]
            ).reshape(4, 32, 256).astype(np.float16),
            "expbt": expBT,
        })

    trace = bool(os.environ.get("KERNEL_TRACE"))
    res = run_bass_kernel_spmd(nc, in_maps, list(range(8)), trace=trace)
    _LAST["exec_time_ns"] = res.exec_time_ns
    _LAST["results"] = res

    parts = [np.asarray(res.results[c]["partial"], np.float32) for c in range(8)]
    out = np.stack([parts[2 * b] + parts[2 * b + 1] + b_out for b in range(B)])
    return out.astype(np.float32)


# revision 47
# speedup vs baseline: 1.0296x; 1.0217x over previous
"""Trainium2 Bass kernel for windowless 3D relative-position attention.

Full-input contract: kernel(**inputs) takes the unsharded numpy inputs and
returns the full [4, 2048, 256] output. Internally shards across 8 NeuronCores
as (batch b = core//2) x (head-group g = core%2, 4 heads each).

v2: ACT-paced softmax pipeline. Per q-chunk (512 queries), the m-loop runs in
(m-block x head-pair) units: 2 row-packed K=32 score matmuls -> one N=1024
ACT exp (PSUM->SBUF fp16) -> one per-m N=2048 DVE multiply by the
host-precomputed exp(bias) -> 2 col-tiled M=33 AV matmuls accumulating
[v|1].T @ aw into per-pair PSUM banks (row 32/96 = softmax denominator).
PSUM budget: 4 banks score double-buffer + 2 AV accumulators + 2 for the
overlapped projection of the previous q-chunk.

Normalization avoids the per-head [1,512] DVE reciprocal (3.3us each): the AV
accumulators are copied to SBUF fp16, the 4 denominator rows are DMA-gathered
onto 4 partitions, and a single [4,512] reciprocal + broadcast-DMA + [32,512]
fp16 multiplies produce the normalized heads, which the output projection
consumes during the next q-chunk.

The bias add is factored through the exponential: exp(s+bias) =
exp(s)*exp(bias), with exp(bias.T - C_SHIFT) precomputed on host in fp16
(C_SHIFT keeps products in fp16 range; it cancels in the softmax ratio).
"""

import os
import sys
from contextlib import ExitStack

import numpy as np

sys.path.insert(0, "/opt/trn_rl_repo")

import concourse.bass as bass
import concourse.bacc as bacc
import concourse.tile as tile
from concourse import mybir
from concourse.bass_utils import run_bass_kernel_spmd

# Problem constants (hardcoded per contract)
B = 4
N = 2048
INP = 256
OUP = 256
HEADS = 8
DIM_HEAD = 32
SCALE = DIM_HEAD ** -0.5
HL = 4            # heads per core
MT = N // 128     # 16 m-tiles (keys)
NQ = 4            # 512-wide n (query) chunks
NQW = 512
C_SHIFT = 4.0

f32 = mybir.dt.float32
f16 = mybir.dt.float16

_LAST = {"exec_time_ns": None}


def _build_nc():
    nc = bacc.Bacc("TRN2", target_bir_lowering=False, debug=False)
    xT_d = nc.dram_tensor("xT", [2, 128, N], f16, kind="ExternalInput")
    wqk_d = nc.dram_tensor("w_qk", [2, 128, 256], f16, kind="ExternalInput")
    wv_d = nc.dram_tensor("w_v", [2, 128, 128], f16, kind="ExternalInput")
    # w_out rows per head, staged host-side as [4, 32, 256]
    wout_d = nc.dram_tensor("w_out4", [4, 32, 256], f16, kind="ExternalInput")
    ebt_d = nc.dram_tensor("expbt", [N, N], f16, kind="ExternalInput")
    out_d = nc.dram_tensor("partial", [N, OUP], f32, kind="ExternalOutput")
    # scratch: denominator gather + reciprocal broadcast staging
    den_d = nc.dram_tensor("den_scratch", [NQ, HL, NQW], f16)
    rec_d = nc.dram_tensor("rec_scratch", [NQ, HL, NQW], f16)

    with ExitStack() as ctx:
        tc = ctx.enter_context(tile.TileContext(nc))
        consts = ctx.enter_context(tc.tile_pool(name="consts", bufs=1))

        xT = consts.tile([128, 2, N], f16)
        wqk = consts.tile([128, 2, 256], f16)
        wv = consts.tile([128, 2, 128], f16)
        # head-pair "slot" layout: slot s holds heads 2s (rows 0:32) and
        # 2s+1 (rows 64:96); softmax denominator rows land on 32 / 96
        wout4 = consts.tile([128, 2, 256], f16)
        qkT = consts.tile([128, 2, N], f16)       # [:,0,:]=qT  [:,1,:]=kT
        vsb = consts.tile([128, MT, HL, 33], f16)  # [m%128, mtile, head, d|ones]
        aoutu = consts.tile([128, NQ, 2, NQW], f16)  # unnormalized AV + dens
        aoutT = consts.tile([128, 2, N], f16)     # normalized heads
        rden = consts.tile([4, NQ, NQW], f16)     # gathered denominators
        rrec = consts.tile([4, NQ, NQW], f16)     # reciprocals
        rb = consts.tile([128, NQ, 2, NQW], f16)  # broadcast reciprocals
        ostg = consts.tile([128, MT, OUP], f32)

        for kk in range(2):
            nc.sync.dma_start(out=xT[:, kk, :], in_=xT_d[kk])
            nc.sync.dma_start(out=wqk[:, kk, :], in_=wqk_d[kk])
            nc.sync.dma_start(out=wv[:, kk, :], in_=wv_d[kk])
        # zero-fill BEFORE the DMAs so the whole-tile wout4c copy below reads
        # fully-initialized memory without clobbering the weights
        nc.vector.memset(wout4[:], 0.0)
        for hl in range(HL):
            nc.sync.dma_start(
                out=wout4[64 * (hl % 2):64 * (hl % 2) + 32, hl // 2, :],
                in_=wout_d[hl],
            )
        nc.vector.memset(vsb[:], 1.0)

        # --- q/k projection (transposed orientation) and v (natural) ---
        with tc.tile_pool(name="pps", bufs=4, space="PSUM") as pps:
            for mb in range(2):           # 0 -> q block, 1 -> k block
                for ch in range(NQ):
                    ps = pps.tile([128, 512], f32, tag="qkps")
                    for kk in range(2):
                        nc.tensor.matmul(
                            ps[:],
                            lhsT=wqk[:, kk, mb * 128:(mb + 1) * 128],
                            rhs=xT[:, kk, ch * 512:(ch + 1) * 512],
                            start=(kk == 0), stop=(kk == 1),
                        )
                    nc.vector.tensor_copy(
                        out=qkT[:, mb, ch * 512:(ch + 1) * 512], in_=ps[:]
                    )
            for nt in range(MT):
                ps = pps.tile([128, 128], f32, tag="vps")
                for kk in range(2):
                    nc.tensor.matmul(
                        ps[:],
                        lhsT=xT[:, kk, nt * 128:(nt + 1) * 128],
                        rhs=wv[:, kk, :],
                        start=(kk == 0), stop=(kk == 1),
                    )
                # scatter the 4 heads' 32 columns into the 33-wide vsb slots;
                # column 32 keeps the memset 1.0 (softmax denominator row)
                nc.vector.tensor_copy(out=vsb[:, nt, :, 0:32], in_=ps[:])

        # wout4c: DVE-owned copy so the projection matmul's weight dep is on
        # DVE (not a DMA sem) — matmuls may carry at most 2 sync waits
        wout4c = consts.tile([128, 2, 256], f16)
        nc.vector.tensor_copy(out=wout4c[:], in_=wout4[:])

        # --- attention: ACT-paced pipeline over (q, m, head-pair) units ---
        def normalize_and_project(prj, q):
            """Batched-reciprocal normalization + output projection for
            chunk q. Issued right after q's m-loop; overlaps chunk q+1."""
            # gather the 4 denominator rows (partitions 32/96 of each pair
            # bank) onto 4 partitions with direct SBUF->SBUF DMAs (no DRAM
            # bounce) — one plain single-partition DMA per head
            for hl in range(HL):
                base, slot = 64 * (hl % 2), hl // 2
                nc.sync.dma_start(
                    out=rden[hl:hl + 1, q, :],
                    in_=aoutu[base + 32:base + 33, q, slot, :],
                )
            with nc.allow_low_precision(reason="fp16 1/den; 5e-4 rel ok"):
                nc.vector.reciprocal(out=rrec[:, q, :], in_=rden[:, q, :])
            nc.sync.dma_start(out=rec_d[q], in_=rrec[:, q, :])
            # broadcast each head's reciprocal to 32 partitions and multiply
            for hl in range(HL):
                base, slot = 64 * (hl % 2), hl // 2
                dst = rec_d[q, hl]
                src_b = bass.AP(
                    tensor=dst.tensor, offset=dst.offset,
                    ap=[[0, 32], dst.ap[-1]],
                )
                nc.sync.dma_start(out=rb[base:base + 32, q, slot, :],
                                  in_=src_b)
            for hl in range(HL):
                base, slot = 64 * (hl % 2), hl // 2
                nc.vector.tensor_mul(
                    aoutT[base:base + 32, slot, q * NQW:(q + 1) * NQW],
                    aoutu[base:base + 32, q, slot, :],
                    rb[base:base + 32, q, slot, :],
                )
            # projection: heads at partition bases 0 vs 64 run in different
            # PE row groups (concurrent!) so they accumulate into separate
            # banks, summed on DVE into the staging tile
            for nb in range(4 * q, 4 * (q + 1)):
                pp = [prj.tile([128, OUP], f32, tag=f"pp{j}",
                               name=f"pp{j}_{nb}") for j in range(2)]
                for hl in range(HL):
                    base, slot = 64 * (hl % 2), hl // 2
                    nc.tensor.matmul(
                        pp[hl % 2][:],
                        lhsT=aoutT[base:base + 32, slot,
                                   nb * 128:(nb + 1) * 128],
                        rhs=wout4c[base:base + 32, slot, :],
                        start=(hl < 2), stop=(hl >= 2),
                        tile_position=(base, 0),
                    )
                # two steps: a TensorTensor may read only ONE input from PSUM
                nc.vector.tensor_copy(out=ostg[:, nb, :], in_=pp[0][:])
                nc.vector.tensor_add(ostg[:, nb, :], ostg[:, nb, :], pp[1][:])
                # stream this n-block's output now — only the last chunk's
                # 512KB remains in the end-of-kernel tail
                nc.sync.dma_start(
                    out=out_d[nb * 128:(nb + 1) * 128, :], in_=ostg[:, nb, :]
                )

        with tc.tile_pool(name="sps", bufs=2, space="PSUM") as sps, \
             tc.tile_pool(name="oap", bufs=1, space="PSUM") as oap, \
             tc.tile_pool(name="prj", bufs=1, space="PSUM") as prj, \
             tc.tile_pool(name="awp", bufs=3) as awp, \
             tc.tile_pool(name="aw2p", bufs=3) as aw2p, \
             tc.tile_pool(name="ebtp", bufs=3) as ebtp:
            for q in range(NQ):
                ncol0 = q * NQW
                oa = [oap.tile([128, NQW], f32, tag=f"oa{i}",
                               name=f"oa{i}_{q}") for i in range(2)]
                # zero the pair banks; AV matmuls accumulate with
                # start=False, which is order-free regardless of stale
                # per-element has_written state (add-onto-0 == overwrite)
                for slot in range(2):
                    nc.vector.memset(oa[slot][:], 0.0)
                for m in range(MT):
                    ebt = ebtp.tile([128, NQW], f16)
                    nc.sync.dma_start(
                        out=ebt[:],
                        in_=ebt_d[m * 128:(m + 1) * 128, ncol0:ncol0 + NQW],
                    )
                    aw = awp.tile([128, HL, NQW], f16)
                    for hp in range(2):
                        # 2 heads row-packed; separate PSUM banks per head
                        sc = sps.tile([128, 2, NQW], f32)
                        for hi in range(2):
                            hl = hp * 2 + hi
                            nc.tensor.matmul(
                                sc[:, hi, :],
                                lhsT=qkT[32 * hl:32 * (hl + 1), 1,
                                         m * 128:(m + 1) * 128],
                                rhs=qkT[32 * hl:32 * (hl + 1), 0,
                                        ncol0:ncol0 + NQW],
                                start=True, stop=True,
                                tile_position=(32 * hl, 0),
                            )
                        nc.scalar.activation(
                            out=aw[:, 2 * hp:2 * hp + 2, :], in_=sc[:],
                            func=mybir.ActivationFunctionType.Exp,
                            scale=SCALE,
                        )
                    # one wide fp16 multiply for all 4 heads of this m-block
                    eb_b = bass.AP(
                        tensor=ebt.tensor, offset=ebt.offset,
                        ap=[ebt.ap[0], [0, HL], ebt.ap[1]],
                    )
                    aw2 = aw2p.tile([128, HL, NQW], f16)
                    nc.vector.tensor_mul(aw2[:], aw[:], eb_b)
                    # AV: col-tiled M=33 per head; pair (2s, 2s+1) lands on
                    # partitions 0:33 / 64:97 of slot s's bank (row 32/96 =
                    # softmax denominator via the vsb ones column)
                    for hl in range(HL):
                        base, slot = 64 * (hl % 2), hl // 2
                        nc.tensor.matmul(
                            oa[slot][base:base + 33, :],
                            lhsT=vsb[:, m, hl, :],
                            rhs=aw2[:, hl, :],
                            start=False, stop=(m == MT - 1),
                            skip_group_check=True,
                            tile_position=(0, base),
                        )
                # stash unnormalized AV output (frees the oa banks)
                for slot in range(2):
                    for base in (0, 64):
                        nc.vector.tensor_copy(
                            out=aoutu[base:base + 33, q, slot, :],
                            in_=oa[slot][base:base + 33, :],
                        )
                normalize_and_project(prj, q)
    nc.compile()
    return nc


_NC_CACHE = {}


def kernel(x, w_qkv, bias_table, w_out, b_out, relative_pos):
    x = np.asarray(x, np.float32)
    w_qkv = np.asarray(w_qkv, np.float32)
    bias_table = np.asarray(bias_table, np.float32)
    w_out = np.asarray(w_out, np.float32)
    b_out = np.asarray(b_out, np.float32)
    relative_pos = np.asarray(relative_pos, np.int32)

    bias = bias_table[relative_pos, 0]                       # [n, m]
    expBT = np.exp(bias.T - C_SHIFT).astype(np.float16)      # [m, n]
    expBT = np.ascontiguousarray(expBT)

    if "nc" not in _NC_CACHE:
        _NC_CACHE["nc"] = _build_nc()
    nc = _NC_CACHE["nc"]

    in_maps = []
    for c in range(8):
        b, g = c // 2, c % 2
        w_qk = np.concatenate(
            [w_qkv[:, g * 128:(g + 1) * 128],
             w_qkv[:, 256 + g * 128:256 + (g + 1) * 128]], axis=1)
        in_maps.append({
            "xT": np.ascontiguousarray(x[b].T).reshape(2, 128, N).astype(np.float16),
            "w_qk": np.ascontiguousarray(w_qk).reshape(2, 128, 256).astype(np.float16),
            "w_v": np.ascontiguousarray(
                w_qkv[:, 512 + g * 128:512 + (g + 1) * 128]
            ).reshape(2, 128, 128).astype(np.float16),
            "w_out4": np.ascontiguousarray(
                w_out[g * 128:(g + 1) * 128, :]
            ).reshape(4, 32, 256).astype(np.float16),
            "expbt": expBT,
        })

    trace = bool(os.environ.get("KERNEL_TRACE"))
    res = run_bass_kernel_spmd(nc, in_maps, list(range(8)), trace=trace)
    _LAST["exec_time_ns"] = res.exec_time_ns
    _LAST["results"] = res

    parts = [np.asarray(res.results[c]["partial"], np.float32) for c in range(8)]
    out = np.stack([parts[2 * b] + parts[2 * b + 1] + b_out for b in range(B)])
    return out.astype(np.float32)
